# revision 1
# baseline (speedup 1.0000x reference)
"""Trainium2 Bass/Tile kernel for a ViT block with routed sparse attention.

v2: fp8 (e4m3) DoubleRow matmuls for all large GEMMs; the routed-neighbor
count matrix enters the scores as ln(C) via two scaled-identity DoubleRow
matmuls accumulated into the scores PSUM (masked entries at -8); softmax
normalization via reciprocal + tensor_tensor_reduce with a stride-0
broadcast; two-term (hi + unscaled-residual lo) fp8 weights for the MLPs
recover most of the fp8 quantization error at 2x (not 4x) the matmul cost.

Layout (per core, B_L=4 batches, T=788 tokens):
  - residual stream x: fp32 SBUF tiles at 64x scale, per-batch subtiles
    (197 = 128+69).
  - activations transposed to [D, T] as fp8 "pair" tiles [128, 2, T]
    (slots = adjacent 128-row k-tiles) feeding DoubleRow directly.
  - Q/K weights column-permuted so evictions land in [32, 2, T]-per-head
    DoubleRow layout for the scores matmul.
"""

import threading
from contextlib import ExitStack

import ml_dtypes
import numpy as np

import concourse.bass as bass  # noqa: F401  (engine types via bacc)
import concourse.bacc as bacc
import concourse.mybir as mybir
import concourse.tile as tile
from concourse.bass_utils import run_bass_kernel_spmd

AF = mybir.ActivationFunctionType
ALU = mybir.AluOpType
PM = mybir.MatmulPerfMode
dt = mybir.dt
BF16 = ml_dtypes.bfloat16
F8 = ml_dtypes.float8_e4m3

B, S, D, H = 32, 197, 768, 12
PN, KN, HD, DFF = 196, 32, 64, 3072
NCORES = 8
BL = B // NCORES
T = BL * S  # 788
SCALE = HD**-0.5

WSQK = 16.0  # q/k weight scale (scores psum = 256*s)
WS = 64.0  # v/p/w1/w2 weight scale == residual stream scale
ALPHA = SCALE / (WSQK * WSQK)
EYE_C = 224.0  # lnC identity scale (2 matmuls)
LNC_B = 1.0 / (ALPHA * 2 * EYE_C)  # lnC storage scale
LNC_MASK = -8.0

MLP1_TERMS = 2  # fp8 weight terms for mlp w1 (1 or 2)
MLP2_TERMS = 1  # fp8 weight terms for mlp w2 (1 or 2)

# per-batch subtiles of the token axis; pair tiles pad each batch to 200
# cols so every DoubleRow lhsT slice offset/stride is 4-byte aligned
TP = 200
TT = 4 * TP
BTS = [(b * TP + o, sz) for b in range(BL) for (o, sz) in ((0, 128), (128, 69))]
BTS_DRAM = [(b * S + o, sz) for b in range(BL) for (o, sz) in ((0, 128), (128, 69))]
TCH = [(0, 400), (400, 400)]  # padded T chunks (= batches 01 / 23)
DCH = [(0, 512), (512, 256)]  # column chunks for [tokens, cols] psum
SST = [(0, 128), (128, 69)]  # s / q tiles inside one batch


def _emit(nc, use_bp, use_b2):
    f32, bf16, f8 = dt.float32, dt.bfloat16, dt.float8e4
    NT1, NT2 = MLP1_TERMS, MLP2_TERMS
    x_d = nc.dram_tensor("x", [T, D], f32, kind="ExternalInput")
    wqk_d = nc.dram_tensor("wqk", [128, 3, 2, 2 * D], f8, kind="ExternalInput")
    wv_d = nc.dram_tensor("wv", [128, 3, 2, D], f8, kind="ExternalInput")
    wp_d = nc.dram_tensor("wp", [128, 3, 2, D], f8, kind="ExternalInput")
    w1_d = nc.dram_tensor("w1", [128, NT1 * 3, 2, DFF], f8, kind="ExternalInput")
    w2_d = nc.dram_tensor("w2", [128, NT2 * 12, 2, D], f8, kind="ExternalInput")
    lnc_d = nc.dram_tensor("lnc", [128, 2, S], f8, kind="ExternalInput")
    eyesc_d = nc.dram_tensor("eyesc", [128, 2, 2, 128], f8, kind="ExternalInput")
    eye8_d = nc.dram_tensor("eye8", [128, 128], f8, kind="ExternalInput")
    bqk_d = nc.dram_tensor("bqk", [128, 12], f32, kind="ExternalInput")
    b1_d = nc.dram_tensor("b1", [128, 24], f32, kind="ExternalInput")
    bp_d = nc.dram_tensor("bp", [1, D], bf16, kind="ExternalInput")
    b2_d = nc.dram_tensor("b2", [1, D], bf16, kind="ExternalInput")
    out_d = nc.dram_tensor("out", [T, D], bf16, kind="ExternalOutput")

    EVE = None  # set inside context

    with tile.TileContext(nc, pool_alloc_mode="queue") as tc, ExitStack() as ctx:
        const = ctx.enter_context(tc.tile_pool(name="const", bufs=1))
        px = ctx.enter_context(tc.tile_pool(name="px", bufs=8))
        pout = ctx.enter_context(tc.tile_pool(name="pout", bufs=8))
        pzn = ctx.enter_context(tc.tile_pool(name="pzn", bufs=3))
        pzt = ctx.enter_context(tc.tile_pool(name="pzt", bufs=6))  # z1t + z2t pairs
        pqk = ctx.enter_context(tc.tile_pool(name="pqk", bufs=6))
        pv = ctx.enter_context(tc.tile_pool(name="pv", bufs=8))
        pew = ctx.enter_context(tc.tile_pool(name="pew", bufs=3))
        patt = ctx.enter_context(tc.tile_pool(name="patt", bufs=3))
        pht = ctx.enter_context(tc.tile_pool(name="pht", bufs=12))
        psm = ctx.enter_context(tc.tile_pool(name="psm", bufs=4))
        paq = ctx.enter_context(tc.tile_pool(name="paq", bufs=6))
        # weight pools, manually released (reverse alloc order)
        pw2s = tc.alloc_tile_pool(name="pw2s", bufs=NT2 * 12)
        pw1s = tc.alloc_tile_pool(name="pw1s", bufs=NT1 * 3)
        pwp = tc.alloc_tile_pool(name="pwp", bufs=3)
        pwv = tc.alloc_tile_pool(name="pwv", bufs=3)
        pwqk = tc.alloc_tile_pool(name="pwqk", bufs=3)

        engs = [nc.vector, nc.scalar]

        def ev_copy(idx, out, in_):
            e = engs[idx % 2]
            if e is nc.scalar:
                e.copy(out=out, in_=in_)
            else:
                e.tensor_copy(out=out, in_=in_)

        # ---- DMAs in consumption order ----
        xts = []
        for i, (o, sz) in enumerate(BTS_DRAM):
            x_t = px.tile([128, D], f32, name=f"x{i}", tag="x", bufs=8)
            xts.append(x_t)
        for i, (o, sz) in enumerate(BTS_DRAM[:4]):
            nc.sync.dma_start(out=xts[i][:sz, 0:512], in_=x_d[o : o + sz, 0:512])
            nc.sync.dma_start(out=xts[i][:sz, 512:D], in_=x_d[o : o + sz, 512:D])
        eye8 = const.tile([128, 128], f8, tag="eye8")
        nc.sync.dma_start(out=eye8, in_=eye8_d[:, :])
        b1_sb = const.tile([128, 24], f32, tag="b1")
        nc.sync.dma_start(out=b1_sb, in_=b1_d[:, :])
        bqk_sb = const.tile([128, 12], f32, tag="bqk")
        nc.sync.dma_start(out=bqk_sb, in_=bqk_d[:, :])
        for i, (o, sz) in enumerate(BTS_DRAM):
            if i < 4:
                continue
            nc.sync.dma_start(out=xts[i][:sz, 0:512], in_=x_d[o : o + sz, 0:512])
            nc.sync.dma_start(out=xts[i][:sz, 512:D], in_=x_d[o : o + sz, 512:D])
        wqk_sb = []
        for kk in range(3):
            w_t = pwqk.tile([128, 2, 2 * D], f8, name=f"wqk{kk}", tag="wqk", bufs=3)
            nc.sync.dma_start(out=w_t, in_=wqk_d[:, kk, :, :])
            wqk_sb.append(w_t)
        wv_sb = []
        for kk in range(3):
            w_t = pwv.tile([128, 2, D], f8, name=f"wv{kk}", tag="wv", bufs=3)
            nc.sync.dma_start(out=w_t, in_=wv_d[:, kk, :, :])
            wv_sb.append(w_t)
        lnc_sb = const.tile([128, 2, S], f8, tag="lnc")
        nc.sync.dma_start(out=lnc_sb, in_=lnc_d[:, :, :])
        eyesc = const.tile([128, 2, 2, 128], f8, tag="eyesc")
        nc.sync.dma_start(out=eyesc, in_=eyesc_d[:, :, :, :])
        wp_sb = []
        for kk in range(3):
            w_t = pwp.tile([128, 2, D], f8, name=f"wp{kk}", tag="wp", bufs=3)
            nc.sync.dma_start(out=w_t, in_=wp_d[:, kk, :, :])
            wp_sb.append(w_t)
        w1_sb = []
        for kk in range(NT1 * 3):
            w_t = pw1s.tile([128, 2, DFF], f8, name=f"w1_{kk}", tag="w1", bufs=NT1 * 3)
            nc.sync.dma_start(out=w_t, in_=w1_d[:, kk, :, :])
            w1_sb.append(w_t)
        w2_sb = []
        for kk in range(NT2 * 12):
            w_t = pw2s.tile([128, 2, D], f8, name=f"w2_{kk}", tag="w2", bufs=NT2 * 12)
            nc.sync.dma_start(out=w_t, in_=w2_d[:, kk, :, :])
            w2_sb.append(w_t)
        if use_bp:
            bp_sb = const.tile([1, D], bf16, tag="bp")
            nc.sync.dma_start(out=bp_sb, in_=bp_d[:, :])
        if use_b2:
            b2_sb = const.tile([1, D], bf16, tag="b2")
            nc.sync.dma_start(out=b2_sb, in_=b2_d[:, :])
        ones_sb = const.tile([1, 128], bf16, tag="ones")
        nc.vector.memset(ones_sb, 1.0)
        eps_sb = const.tile([128, 1], f32, tag="eps")
        nc.vector.memset(eps_sb, 1e-5)
        # pre-touch an ACT-consumed DMA tile + warm the ln/exp table
        act_touch = const.tile([128, 1], f32, tag="act_touch")
        nc.scalar.copy(out=act_touch, in_=b1_sb[:, 0:1])
        act_warm = const.tile([128, 1], f32, tag="act_warm")
        nc.scalar.activation(out=act_warm, in_=eps_sb, func=AF.Exp)

        # ---- LN (stats DVE; rstd via Ln/Exp; apply -> fp8 zn) ----
        def layer_norm(i, mv_pool_tag):
            o, sz = BTS[i]
            stats = psm.tile([128, 2, 6], f32, tag="stats", bufs=4)
            for g in range(2):
                nc.vector.bn_stats(
                    out=stats[:sz, g, :], in_=xts[i][:sz, g * 384 : (g + 1) * 384]
                )
            mv = psm.tile([128, 2], f32, tag=mv_pool_tag, bufs=4)
            nc.vector.bn_aggr(out=mv[:sz], in_=stats[:sz])
            nc.scalar.activation(
                out=mv[:sz, 1:2], in_=mv[:sz, 1:2], func=AF.Ln,
                bias=eps_sb[:sz], scale=1.0,
            )
            nc.scalar.activation(
                out=mv[:sz, 1:2], in_=mv[:sz, 1:2], func=AF.Exp, scale=-0.5,
            )
            zn = pzn.tile([128, D], f8, tag="zn", bufs=3)
            nc.gpsimd.tensor_scalar(
                out=zn[:sz, :], in0=xts[i][:sz, :],
                scalar1=mv[:sz, 0:1], scalar2=mv[:sz, 1:2],
                op0=ALU.subtract, op1=ALU.mult,
            )
            return zn

        def transpose_tile(i, zn, zt_pairs, tr_pool, ev_base):
            o, sz = BTS[i]
            for kk in range(3):
                trp = tr_pool.tile([128, 2, 128, 2], f8, tag="tr", bufs=2)
                for half in range(2):
                    k = 2 * kk + half
                    nc.tensor.transpose(
                        out=trp[:, half, :sz, 0],
                        in_=zn[:sz, k * 128 : (k + 1) * 128],
                        identity=eye8[:sz, :sz],
                    )
                ev_copy(ev_base + kk, zt_pairs[kk][:, :, o : o + sz], trp[:, :, :sz, 0])

        # ---- Phase A: LN1, z1 transposes, QKV ----
        ps_tr = tc.alloc_tile_pool(name="ps_tr", bufs=2, space="PSUM")
        ps_mm = tc.alloc_tile_pool(name="ps_mm", bufs=3, space="PSUM")
        z1t = [pzt.tile([128, 2, TT], f8, name=f"z1t{kk}", tag="zt", bufs=6) for kk in range(3)]
        for kk in range(3):
            nc.gpsimd.memset(
                z1t[kk].rearrange("p a (b c) -> p a b c", c=TP)[:, :, :, S:], 0.0
            )
        qt = [pqk.tile([128, 2, TT], f8, name=f"qt{i}", tag="qk", bufs=6) for i in range(3)]
        kt = [pqk.tile([128, 2, TT], f8, name=f"kt{i}", tag="qk", bufs=6) for i in range(3)]
        vaug = [pv.tile([128, H, 65], bf16, name=f"v{i}", tag="v", bufs=8) for i in range(8)]

        def v_subtile(i, ev_base):
            o, sz = BTS[i]
            va = vaug[i]
            nc.vector.memset(va[:sz, :, 64:65], 1.0)
            for ci, (off, cs) in enumerate(DCH):
                mm = ps_mm.tile([128, 512], f32, tag="mm", bufs=3)
                for kk in range(3):
                    nc.tensor.matmul(
                        mm[:sz, :cs],
                        z1t[kk][:, :, o : o + sz],
                        wv_sb[kk][:, :, off : off + cs],
                        start=(kk == 0), stop=(kk == 2),
                        perf_mode=PM.DoubleRow,
                    )
                e = engs[(ev_base + ci) % 2]
                vout = va[:sz, off // 64 : (off + cs) // 64, 0:64]
                vin = mm[:sz, :cs].rearrange("p (h c) -> p h c", c=64)
                if e is nc.scalar:
                    e.activation(out=vout, in_=vin, func=AF.Copy,
                                 bias=0.0, scale=1.0 / WS)
                else:
                    e.tensor_scalar(out=vout, in0=vin, scalar1=1.0 / WS,
                                    scalar2=0.0, op0=ALU.mult, op1=ALU.add)

        def qk_chunk(ci, ev_base):
            off, cs = TCH[ci]
            for blk in range(12):
                i, s = (blk % 6) // 2, blk % 2
                dst = qt[i] if blk < 6 else kt[i]
                mm = ps_mm.tile([128, 512], f32, tag="mm", bufs=3)
                for kk in range(3):
                    nc.tensor.matmul(
                        mm[:, :cs],
                        wqk_sb[kk][:, :, blk * 128 : (blk + 1) * 128],
                        z1t[kk][:, :, off : off + cs],
                        start=(kk == 0), stop=(kk == 2),
                        perf_mode=PM.DoubleRow,
                    )
                e = engs[(ev_base + blk) % 2]
                if e is nc.scalar:
                    e.activation(
                        out=dst[:, s, off : off + cs], in_=mm[:, :cs],
                        func=AF.Identity, bias=bqk_sb[:, blk : blk + 1], scale=1.0,
                    )
                else:
                    e.tensor_scalar(
                        out=dst[:, s, off : off + cs],
                        in0=mm[:, :cs],
                        scalar1=bqk_sb[:, blk : blk + 1], scalar2=0.0,
                        op0=ALU.add, op1=ALU.add,
                    )

        for i in range(4):
            zn = layer_norm(i, "mv1")
            transpose_tile(i, zn, z1t, ps_tr, i * 3)
        qk_chunk(0, 0)
        v_subtile(0, 0)
        v_subtile(1, 2)
        for i in range(4, 8):
            zn = layer_norm(i, "mv1")
            transpose_tile(i, zn, z1t, ps_tr, i * 3)
        qk_chunk(1, 1)
        v_subtile(2, 0)
        v_subtile(3, 2)
        v_subtile(4, 1)
        v_subtile(5, 0)
        v_subtile(6, 2)
        v_subtile(7, 1)
        ps_mm.release()
        ps_tr.release()

        # ---- Phase B: attention (+ proj/LN2 per completed batch) ----
        ps_sc = tc.alloc_tile_pool(name="ps_sc", bufs=2, space="PSUM")
        ps_av = tc.alloc_tile_pool(name="ps_av", bufs=2, space="PSUM")
        ps_atr = tc.alloc_tile_pool(name="ps_atr", bufs=1, space="PSUM")
        ps_pj = tc.alloc_tile_pool(name="ps_pj", bufs=1, space="PSUM")
        attp = [patt.tile([128, 2, TT], f8, name=f"attp{kk}", tag="attp", bufs=3) for kk in range(3)]
        z2t = [pzt.tile([128, 2, TT], f8, name=f"z2t{kk}", tag="zt", bufs=6) for kk in range(3)]
        for kk in range(3):
            nc.gpsimd.memset(
                z2t[kk].rearrange("p a (b c) -> p a b c", c=TP)[:, :, :, S:], 0.0
            )

        def attention(b, j):
            bo = b * TP
            scp = ps_sc.tile([128, 2, 2, 256], f32, tag="sc", bufs=2)
            for hh in range(2):
                h = 2 * j + hh
                i, band = h // 4, 32 * (h % 4)
                for st, (so, ss) in enumerate(SST):
                    nc.tensor.matmul(
                        scp[:ss, hh, st, :S],
                        kt[i][band : band + 32, :, bo + so : bo + so + ss],
                        qt[i][band : band + 32, :, bo : bo + S],
                        start=True, stop=False, perf_mode=PM.DoubleRow,
                        tile_position=(band, 0),
                    )
                    for rep in range(2):
                        nc.tensor.matmul(
                            scp[:ss, hh, st, :S],
                            eyesc[:, st, :, :ss],
                            lnc_sb[:, :, :],
                            start=False, stop=(rep == 1),
                            perf_mode=PM.DoubleRow,
                        )
            ew = pew.tile([128, 2, 2, S], bf16, tag="ew", bufs=3)
            nc.scalar.activation(
                out=ew[:, :, 0, :], in_=scp[:, :, 0, :S], func=AF.Exp, scale=ALPHA
            )
            nc.scalar.activation(
                out=ew[:69, :, 1, :], in_=scp[:69, :, 1, :S], func=AF.Exp, scale=ALPHA
            )
            atr = ps_atr.tile([128, S, 2], f8, tag="atr", bufs=1)
            av = ps_av.tile([128, 2, 2, 65], f32, tag="av", bufs=2)
            for qi, (qo, qs) in enumerate(SST):
                for hh in range(2):
                    h = 2 * j + hh
                    for st, (so, ss) in enumerate(SST):
                        nc.tensor.matmul(
                            av[:qs, qi, hh, :],
                            ew[:ss, hh, st, qo : qo + qs],
                            vaug[2 * b + st][:ss, h, :],
                            start=(st == 0), stop=(st == 1),
                        )
            # one recip + one normalize for both q-tiles (rows 69:128 of the
            # 69-row q-tile hold stale psum; their outputs are never read)
            rec = psm.tile([128, 2, 2], f32, tag="rec", bufs=4)
            nc.vector.reciprocal(out=rec, in_=av[:, :, :, 64])
            aq = paq.tile([128, 2, 2, 64], f8, tag="aq", bufs=6)
            nc.vector.tensor_tensor(
                out=aq, in0=av[:, :, :, 0:64],
                in1=rec[:, :, :, None].broadcast_to((128, 2, 2, 64)),
                op=ALU.mult,
            )
            for qi, (qo, qs) in enumerate(SST):
                nc.tensor.transpose(
                    out=atr[:, qo : qo + qs, 0],
                    in_=aq[:qs, qi].rearrange("p a b -> p (a b)"),
                    identity=eye8[:qs, :qs],
                )
            ev_copy(b + j, attp[j // 2][:, j % 2, bo : bo + S], atr[:, :, 0])

        def proj_ln2(b, do_ln=True):
            for st in range(2):
                i = 2 * b + st
                o, sz = BTS[i]
                for ci, (off, cs) in enumerate(DCH):
                    mm = ps_pj.tile([128, 512], f32, tag="pj", bufs=1)
                    for kk in range(3):
                        nc.tensor.matmul(
                            mm[:sz, :cs],
                            attp[kk][:, :, o : o + sz],
                            wp_sb[kk][:, :, off : off + cs],
                            start=(kk == 0), stop=(kk == 2 and not use_bp),
                            perf_mode=PM.DoubleRow,
                        )
                    if use_bp:
                        nc.tensor.matmul(
                            mm[:sz, :cs],
                            ones_sb[0:1, :sz],
                            bp_sb[0:1, off : off + cs],
                            start=False, stop=True,
                        )
                    nc.vector.scalar_tensor_tensor(
                        out=xts[i][:sz, off : off + cs],
                        in0=mm[:sz, :cs], scalar=1.0 / WS,
                        in1=xts[i][:sz, off : off + cs],
                        op0=ALU.mult, op1=ALU.add,
                    )
                if do_ln:
                    zn2s[i] = layer_norm(i, "mv2")


        zn2s = {}
        for b in range(BL):
            for j in range(6):
                attention(b, j)
            proj_ln2(b, do_ln=(b < 2))
        ps_pj.release()
        ps_atr.release()
        ps_av.release()
        ps_sc.release()
        pwqk.release()
        pwv.release()
        pwp.release()

        # ---- Phase C: z2 transposes + MLP1 ----
        ps_m1 = tc.alloc_tile_pool(name="ps_m1", bufs=3, space="PSUM")
        ps_tr2 = tc.alloc_tile_pool(name="ps_tr2", bufs=2, space="PSUM")
        for i in range(4):
            transpose_tile(i, zn2s[i], z2t, ps_tr2, i)

        ht = [pht.tile([128, 2, TT], f8, name=f"ht{kk}", tag="ht", bufs=12) for kk in range(12)]

        def mlp1_ft_ch(ft, ci):
            off, cs = TCH[ci]
            mm = ps_m1.tile([128, 512], f32, tag="m1", bufs=3)
            nmm = NT1 * 3
            for t in range(nmm):
                kk = t % 3
                nc.tensor.matmul(
                    mm[:, :cs],
                    w1_sb[t][:, :, ft * 128 : (ft + 1) * 128],
                    z2t[kk][:, :, off : off + cs],
                    start=(t == 0), stop=(t == nmm - 1),
                    perf_mode=PM.DoubleRow,
                )
            nc.scalar.activation(
                out=ht[ft // 2][:, ft % 2, off : off + cs],
                in_=mm[:, :cs],
                func=AF.Gelu,
                bias=b1_sb[:, ft : ft + 1],
                scale=1.0 / WS,
            )

        for ft in range(24):
            mlp1_ft_ch(ft, 0)
            if ft == 0:
                for i in range(4, 8):
                    zn2s[i] = layer_norm(i, "mv2")
                    transpose_tile(i, zn2s[i], z2t, ps_tr2, i)
        ps_tr2.release()

        # ---- Phase D: MLP2 (batches 01 interleave with MLP1 ch1) ----
        ps_m2 = tc.alloc_tile_pool(name="ps_m2", bufs=3, space="PSUM")

        def mlp2_subtile(i):
            o, sz = BTS_DRAM[i]
            out_t = pout.tile([128, D], bf16, name=f"o{i}", tag="out", bufs=8)
            for ci, (off, cs) in enumerate(DCH):
                mm = ps_m2.tile([128, 512], f32, tag="m2", bufs=3)
                nmm = NT2 * 12
                for t in range(nmm):
                    kk = t % 12
                    nc.tensor.matmul(
                        mm[:sz, :cs],
                        ht[kk][:, :, BTS[i][0] : BTS[i][0] + sz],
                        w2_sb[t][:, :, off : off + cs],
                        start=(t == 0), stop=(t == nmm - 1 and not use_b2),
                        perf_mode=PM.DoubleRow,
                    )
                if use_b2:
                    nc.tensor.matmul(
                        mm[:sz, :cs],
                        ones_sb[0:1, :sz],
                        b2_sb[0:1, off : off + cs],
                        start=False, stop=True,
                    )
                nc.vector.scalar_tensor_tensor(
                    out=out_t[:sz, off : off + cs],
                    in0=mm[:sz, :cs], scalar=1.0 / WS,
                    in1=xts[i][:sz, off : off + cs],
                    op0=ALU.mult, op1=ALU.add,
                )
                nc.sync.dma_start(
                    out=out_d[o : o + sz, off : off + cs],
                    in_=out_t[:sz, off : off + cs],
                )

        for ft in range(24):
            mlp1_ft_ch(ft, 1)
            if ft % 6 == 2:
                mlp2_subtile(ft // 6)
        for i in range(4, 8):
            mlp2_subtile(i)
        ps_m2.release()
        ps_m1.release()
        pw1s.release()
        pw2s.release()

    return nc


_nc_lock = threading.Lock()
_nc_cache = {}


def _constrain_act_tables():
    import concourse.hw_specs as hw_specs

    orig = hw_specs.get_activation_tables
    keep = {"natural_log_exp_and_others", "gelu_and_others"}

    def patched(arch):
        tabs = orig(arch)
        return {k: (set(v) if k in keep else set()) for k, v in tabs.items()}

    bacc.get_activation_tables = patched


def _get_nc(use_bp=False, use_b2=False):
    key = (use_bp, use_b2, MLP1_TERMS, MLP2_TERMS)
    with _nc_lock:
        if key not in _nc_cache:
            _constrain_act_tables()
            nc = bacc.Bacc("TRN2", target_bir_lowering=False)
            with nc.allow_low_precision(reason="fp8 kernel"):
                _emit(nc, use_bp, use_b2)
            nc.finalize()
            _nc_cache[key] = nc
        return _nc_cache[key]


def _f8(a):
    return np.clip(np.asarray(a, np.float32), -240.0, 240.0).astype(F8)


def _pack_dr(w, ncols):
    """[K, ncols] -> [128, K//256, 2, ncols] DoubleRow pairs."""
    k = w.shape[0]
    return np.ascontiguousarray(
        w.reshape(k // 256, 2, 128, ncols).transpose(2, 0, 1, 3)
    )


def _prep_inputs(inputs):
    x = np.asarray(inputs["x"], np.float32)
    routes = np.asarray(inputs["routes"], np.int64)
    qkv_w = np.asarray(inputs["qkv_w"], np.float32)
    qkv_b = np.asarray(inputs["qkv_b"], np.float32)
    proj_w = np.asarray(inputs["proj_w"], np.float32)
    proj_b = np.asarray(inputs["proj_b"], np.float32)
    n1_g = np.asarray(inputs["n1_g"], np.float32)
    n1_b = np.asarray(inputs["n1_b"], np.float32)
    n2_g = np.asarray(inputs["n2_g"], np.float32)
    n2_b = np.asarray(inputs["n2_b"], np.float32)
    mlp_w1 = np.asarray(inputs["mlp_w1"], np.float32)
    mlp_b1 = np.asarray(inputs["mlp_b1"], np.float32)
    mlp_w2 = np.asarray(inputs["mlp_w2"], np.float32)
    mlp_b2 = np.asarray(inputs["mlp_b2"], np.float32)

    # Q/K column permutation: block blk=2i+s holds heads 4i..4i+3, dims 32s:32s+32
    perm = np.concatenate(
        [
            np.arange((4 * i + j) * 64 + 32 * s, (4 * i + j) * 64 + 32 * s + 32)
            for i in range(3)
            for s in range(2)
            for j in range(4)
        ]
    )
    wq = qkv_w[:, :D] * n1_g[:, None]
    wk = qkv_w[:, D : 2 * D] * n1_g[:, None]
    wv = qkv_w[:, 2 * D :] * n1_g[:, None]
    wqk_perm = np.concatenate([wq[:, perm], wk[:, perm]], axis=1)
    wqk8 = _pack_dr(_f8(WSQK * wqk_perm), 2 * D)
    wv8 = _pack_dr(_f8(WS * wv), D)
    wp8 = _pack_dr(_f8(WS * proj_w), D)

    def two_term(w, nterms):
        w64 = WS * w
        hi = _f8(w64)
        if nterms == 1:
            return _pack_dr(hi, w.shape[1])
        lo = _f8(w64 - hi.astype(np.float32))
        return np.concatenate(
            [_pack_dr(hi, w.shape[1]), _pack_dr(lo, w.shape[1])], axis=1
        )

    w18 = two_term(mlp_w1 * n2_g[:, None], MLP1_TERMS)
    w28 = two_term(mlp_w2, MLP2_TERMS)

    # biases
    bqk_full = n1_b @ qkv_w[:, : 2 * D] + qkv_b[: 2 * D]
    bqk_perm = np.concatenate([bqk_full[:D][perm], bqk_full[D:][perm]])
    bqk = np.ascontiguousarray((WSQK * bqk_perm).reshape(12, 128).T)
    bv = n1_b @ qkv_w[:, 2 * D :] + qkv_b[2 * D :]
    bp = proj_b + bv @ proj_w
    b1 = np.ascontiguousarray(
        (n2_b @ mlp_w1 + mlp_b1).reshape(24, 128).T
    )
    use_bp = bool(np.any(bp != 0))
    use_b2 = bool(np.any(mlp_b2 != 0))

    # count matrix -> ln counts (key-major [s, q]); masked at LNC_MASK
    ct = np.zeros((S, S), np.float32)
    np.add.at(ct, (routes.reshape(-1) + 1, np.repeat(np.arange(PN), KN) + 1), 1.0)
    ct[:, 0] = 1.0
    lnc = np.where(ct > 0, np.log(np.maximum(ct, 1e-9)), LNC_MASK)
    lnc8 = np.zeros((128, 2, S), np.float32)
    lnc8[:, 0, :] = lnc[0:128, :]
    lnc8[:69, 1, :] = lnc[128:S, :]
    lnc8 = _f8(LNC_B * lnc8)
    lnc8[69:, 1, :] = np.float32(0.0).astype(F8)

    eyesc = np.zeros((128, 2, 2, 128), np.float32)
    eyesc[:, 0, 0, :] = EYE_C * np.eye(128)
    eyesc[:69, 1, 1, :69] = EYE_C * np.eye(69)
    eyesc = eyesc.astype(F8)
    eye8 = np.eye(128).astype(F8)

    shared = {
        "wqk": wqk8, "wv": wv8, "wp": wp8, "w1": w18, "w2": w28,
        "lnc": lnc8, "eyesc": eyesc, "eye8": eye8,
        "bqk": bqk.astype(np.float32), "b1": b1.astype(np.float32),
        "bp": (WS * bp).astype(BF16).reshape(1, D),
        "b2": (WS * mlp_b2).astype(BF16).reshape(1, D),
    }
    in_maps = []
    for c in range(NCORES):
        m = dict(shared)
        m["x"] = np.ascontiguousarray(
            x[c * BL : (c + 1) * BL].reshape(T, D).astype(np.float32)
        )
        in_maps.append(m)
    return in_maps, use_bp, use_b2


def run(inputs, trace=False):
    in_maps, use_bp, use_b2 = _prep_inputs(inputs)
    nc = _get_nc(use_bp, use_b2)
    res = run_bass_kernel_spmd(
        nc, in_maps, core_ids=list(range(NCORES)), trace=trace
    )
    out = np.concatenate(
        [np.asarray(r["out"]).astype(np.float32).reshape(BL, S, D) for r in res.results],
        axis=0,
    )
    res.used_nc = nc
    return out, res


def kernel(**inputs):
    out, _ = run(inputs, trace=False)
    return out

